# revision 16
# baseline (speedup 1.0000x reference)
"""Trainium2 Bass kernel for BlockFFTDirectPrior.

Computes out = irfft(einsum('bjn,ijn->bin', rfft(x_blocks), conj(W)))
reshaped to [B, 4096], for x [4096, 4096] f32, W [16, 16, 129] complex
(block size 256).

Strategy: data-parallel over the batch axis across 8 NeuronCores (512 rows
each); W-derived constants replicated. Per core, four PE stages:

  T: transpose x tiles (PE transpose vs identity)      -> xt [t, b] per block
  F: real DFT as matmul (contract t, K=2x128 chunks)   -> X  [n, b] per block
       R0 rows n=0..127 hold Xr[n]; R1 row 0 holds Xr[128] (Nyquist),
       rows p=1..127 hold Xi[p].
  E: per-frequency 16x16 complex mixing as 8-frequency block-diagonal
     matmuls (K = (j,f) = 128)                         -> Y [(i,f), b] per group
  I: real inverse DFT with the data as the stationary operand, which
     restores the [b, m] orientation for free            -> out [b, i*256+m]

Between F/E and E/I, single big SBUF->SBUF DMAs perform the partition
regroupings ((n per j) -> ((j,f) per g), and ((i,f) per g) -> (n per i)).
"""

import os
import numpy as np
from contextlib import ExitStack

import concourse.bass as bass
import concourse.tile as tile
from concourse import bacc, mybir
from concourse.bass_utils import run_bass_kernel_spmd

NCORES = 8
B_FULL, D_IN, D_OUT, BS = 4096, 4096, 4096, 256
BC = B_FULL // NCORES          # 512 batch rows per core
KIN = KOUT = 16
NG = 16                        # groups of 8 frequencies covering n=0..127
F32 = mybir.dt.float32
F32R = mybir.dt.float32r       # single-pass PE fp32 mode (4x faster matmul)

_CACHE = {}
LAST_RESULTS = None            # BassKernelResults of the most recent run


# DFT/IDFT row swizzle: row r = f*16+g holds frequency n = 8g+f. This makes
# both partition regroups plain affine DMAs (partition dim outermost, step 1).
PERM = np.array([8 * (r % 16) + r // 16 for r in range(128)])


def _build_consts(W_real, W_imag):
    """Constant matrices in the exact SBUF layouts the kernel reads."""
    f32 = np.float32
    t = np.arange(BS)
    n0 = np.arange(128)
    ang = 2.0 * np.pi / BS

    CF0 = np.cos(ang * np.outer(t, n0))
    CF1 = np.empty((BS, 128))
    CF1[:, 0] = np.cos(np.pi * t)
    p = np.arange(1, 128)
    CF1[:, 1:] = -np.sin(ang * np.outer(t, p))
    CF0 = CF0[:, PERM]
    CF1 = CF1[:, PERM]
    cfs = np.stack([
        np.concatenate([CF0[:128], CF0[128:]], axis=1),
        np.concatenate([CF1[:128], CF1[128:]], axis=1),
    ], axis=1).astype(f32)                                  # [128, 2, 256]

    # wpk[(j*8+f), g, c, (i*8+f)] = M_c[i, j, 8g+f];  M = (Wr, Wi, -Wi)
    wpk = np.zeros((128, NG, 3, 128), dtype=f32)
    jj = np.arange(KIN)[:, None, None]
    ii = np.arange(KOUT)[None, :, None]
    ff = np.arange(8)[None, None, :]
    for g in range(NG):
        for c, M in enumerate((W_real, W_imag, -W_imag)):
            wpk[jj * 8 + ff, g, c, ii * 8 + ff] = M[ii, jj, 8 * g + ff]
    wnyq = np.ascontiguousarray(W_real[:, :, 128].T).astype(f32)  # [j, i]

    m = np.arange(BS)
    D0 = np.empty((128, BS))
    D0[0] = 1.0 / BS
    nn = np.arange(1, 128)
    D0[1:] = (2.0 / BS) * np.cos(ang * np.outer(nn, m))
    D1 = np.empty((128, BS))
    D1[0] = ((-1.0) ** m) / BS
    D1[1:] = -(2.0 / BS) * np.sin(ang * np.outer(nn, m))
    dmat = np.stack([D0[PERM], D1[PERM]], axis=1).astype(f32)  # [128, 2, 256]

    ident = np.eye(128, dtype=f32)
    return {"cfs": cfs, "wpk": wpk, "wnyq": wnyq, "dmat": dmat, "ident": ident}


def _build_program():
    nc = bacc.Bacc(
        "TRN2", target_bir_lowering=False, debug=False, num_devices=NCORES
    )
    x_d = nc.dram_tensor("x", [BC, D_IN], F32, kind="ExternalInput").ap()
    cfs_d = nc.dram_tensor("cfs", [128, 2, 256], F32R, kind="ExternalInput").ap()
    wpk_d = nc.dram_tensor("wpk", [128, NG, 3, 128], F32R, kind="ExternalInput").ap()
    wnyq_d = nc.dram_tensor("wnyq", [KIN, KOUT], F32R, kind="ExternalInput").ap()
    dmat_d = nc.dram_tensor("dmat", [128, 2, 256], F32R, kind="ExternalInput").ap()
    ident_d = nc.dram_tensor("ident", [128, 128], F32, kind="ExternalInput").ap()
    out_d = nc.dram_tensor("out", [BC, D_OUT], F32, kind="ExternalOutput").ap()

    cp_state = [0]

    with tile.TileContext(nc) as tc, ExitStack() as ctx:
        def copy(dst, src):
            # alternate PSUM->SBUF copies between DVE and ACT
            if cp_state[0] % 2 == 0:
                nc.vector.tensor_copy(dst, src)
            else:
                nc.scalar.copy(dst, src)
            cp_state[0] += 1

        consts = ctx.enter_context(tc.tile_pool(name="consts", bufs=1))
        stg = ctx.enter_context(tc.tile_pool(name="stg", bufs=5))
        ps = ctx.enter_context(tc.tile_pool(name="ps", bufs=6, space="PSUM"))

        cfs = consts.tile([128, 2, 256], F32R)
        wpk = consts.tile([128, NG, 3, 128], F32R)
        wnyq = consts.tile([KIN, KOUT], F32R)
        dmat = consts.tile([128, 2, 256], F32R)
        ident = consts.tile([128, 128], F32)
        gnyq = consts.tile([KIN, BC], F32R)
        ynyq = consts.tile([KOUT, BC], F32R)

        nc.sync.dma_start(cfs[:], cfs_d)
        nc.sync.dma_start(wpk[:], wpk_d)
        nc.sync.dma_start(wnyq[:], wnyq_d)
        nc.sync.dma_start(dmat[:], dmat_d)
        nc.sync.dma_start(ident[:], ident_d)

        # ---- load x: [b, d] in 4 chunks of 128 rows
        xs0 = stg.tile([128, 2, D_IN], F32, tag="stg")
        xs1 = stg.tile([128, 2, D_IN], F32, tag="stg")
        xsv = [xs0, xs1]
        for b4 in range(4):
            nc.sync.dma_start(
                xsv[b4 // 2][:, b4 % 2, :], x_d[128 * b4:128 * (b4 + 1), :]
            )

        # ---- stage T: xt[dc][t_lo, b], dc = j*2 + tc
        xt0 = stg.tile([128, 16, BC], F32R, tag="stg")
        xt1 = stg.tile([128, 16, BC], F32R, tag="stg")
        xtv = [xt0, xt1]
        for bc in range(4):
            for dcg in range(8):
                pt = ps.tile([128, 4, 128], F32, tag="ps")
                for q in range(4):
                    dc = dcg * 4 + q
                    nc.tensor.transpose(
                        pt[:, q, :],
                        xsv[bc // 2][:, bc % 2, 128 * dc:128 * (dc + 1)],
                        ident[:],
                    )
                dst = xtv[dcg // 4][
                    :, 4 * (dcg % 4):4 * (dcg % 4) + 4, 128 * bc:128 * (bc + 1)
                ]
                copy(dst, pt[:])

        # ---- stage F: real DFT (fp32r matmuls); regroup1 DMAs trail per j,
        # alternating issue rings (sync HWDGE / gpsimd SWDGE) for overlap.
        xfr = stg.tile([128, KIN, BC], F32R, tag="stg")
        xfi = stg.tile([128, KIN, BC], F32R, tag="stg")
        ggr = stg.tile([128, NG, BC], F32R, tag="stg")
        ggi = stg.tile([128, NG, BC], F32R, tag="stg")
        for j in range(KIN):
            for which, dstT in ((0, xfr), (1, xfi)):
                pf = ps.tile([128, BC], F32, tag="ps")
                for tc_ in range(2):
                    nc.tensor.matmul(
                        pf[:],
                        cfs[:, which, 128 * tc_:128 * (tc_ + 1)],
                        xtv[j // 8][:, 2 * (j % 8) + tc_, :],
                        start=(tc_ == 0),
                        stop=(tc_ == 1),
                    )
                copy(dstT[:, j, :], pf[:])
            # regroup1: ggr[8j+f, g, b] = xfr[f*16+g, j, b]
            eng = nc.sync if j % 2 == 0 else nc.gpsimd
            eng.dma_start(out=ggr[8 * j:8 * (j + 1), :, :], in_=xfr[:, j, :])
            eng.dma_start(out=ggi[8 * j:8 * (j + 1), :, :], in_=xfi[:, j, :])
        nc.sync.dma_start(out=gnyq[:], in_=xfi[0:1, :, :])

        # ---- stage E: blockdiag einsum (fp32r)
        yyr = stg.tile([128, NG, BC], F32R, tag="stg")
        yyi = stg.tile([128, NG, BC], F32R, tag="stg")
        yh0 = stg.tile([128, KOUT, BC], F32R, tag="stg")
        yh1 = stg.tile([128, KOUT, BC], F32R, tag="stg")
        for g in range(NG):
            pyr = ps.tile([128, BC], F32, tag="ps")
            nc.tensor.matmul(pyr[:], wpk[:, g, 0, :],
                             ggr[:, g, :], start=True, stop=False)
            nc.tensor.matmul(pyr[:], wpk[:, g, 1, :],
                             ggi[:, g, :], start=False, stop=True)
            copy(yyr[:, g, :], pyr[:])
            pyi = ps.tile([128, BC], F32, tag="ps")
            nc.tensor.matmul(pyi[:], wpk[:, g, 0, :],
                             ggi[:, g, :], start=True, stop=False)
            nc.tensor.matmul(pyi[:], wpk[:, g, 2, :],
                             ggr[:, g, :], start=False, stop=True)
            copy(yyi[:, g, :], pyi[:])
        pyn = ps.tile([KIN, BC], F32, tag="ps")
        nc.tensor.matmul(pyn[:], wnyq[:],
                         gnyq[:], start=True, stop=True)
        copy(ynyq[:], pyn[:])
        # regroup2: yh0[f*16+g, i, b] = yyr[8i+f, g, b]; same for yh1/yyi
        for i in range(KOUT):
            eng = nc.sync if i % 2 == 0 else nc.gpsimd
            eng.dma_start(out=yh0[:, i, :], in_=yyr[8 * i:8 * (i + 1), :, :])
            eng.dma_start(out=yh1[:, i, :], in_=yyi[8 * i:8 * (i + 1), :, :])
        # Nyquist goes to row 0 of yh1 (overwrites the meaningless Zi[0] row)
        nc.sync.dma_start(out=yh1[0:1, :, :], in_=ynyq[:])

        # ---- stage I: inverse DFT, data as stationary operand -> [b, m]
        os0 = stg.tile([128, 2, D_OUT], F32, tag="stg")
        os1 = stg.tile([128, 2, D_OUT], F32, tag="stg")
        osv = [os0, os1]
        for bs in range(4):
            for i in range(KOUT):
                po = ps.tile([128, BS], F32, tag="ps")
                nc.tensor.matmul(
                    po[:], yh0[:, i, 128 * bs:128 * (bs + 1)],
                    dmat[:, 0, :], start=True, stop=False)
                nc.tensor.matmul(
                    po[:], yh1[:, i, 128 * bs:128 * (bs + 1)],
                    dmat[:, 1, :], start=False, stop=True)
                copy(osv[bs // 2][:, bs % 2, BS * i:BS * (i + 1)], po[:])
            nc.scalar.dma_start(
                out_d[128 * bs:128 * (bs + 1), :], osv[bs // 2][:, bs % 2, :]
            )

    nc.compile()
    return nc


def _get_program():
    if "nc" not in _CACHE:
        _CACHE["nc"] = _build_program()
    return _CACHE["nc"]


def _install_ntff_hook():
    """Provide antenv.axon_hooks (absent in this image) so that
    run_bass_kernel_spmd(trace=True) can capture NTFF profiles through the
    axon client library."""
    import sys
    import types
    import ctypes
    import contextlib

    if "antenv.axon_hooks" in sys.modules:
        return
    try:
        lib = ctypes.CDLL("/opt/axon/libaxon_pjrt.so")
    except OSError:
        return
    if not hasattr(lib, "axon_start_nrt_profile"):
        return
    lib.axon_start_nrt_profile.argtypes = [
        ctypes.POINTER(ctypes.c_int64),
        ctypes.c_size_t,
    ]
    lib.axon_start_nrt_profile.restype = ctypes.c_int64
    lib.axon_stop_nrt_profile.argtypes = [ctypes.c_char_p]
    lib.axon_stop_nrt_profile.restype = ctypes.c_int64

    @contextlib.contextmanager
    def _hook(output_dir, device_ids):
        import jax

        jax.devices()
        if device_ids:
            ids = (ctypes.c_int64 * len(device_ids))(*device_ids)
            rc = lib.axon_start_nrt_profile(ids, len(device_ids))
        else:
            rc = lib.axon_start_nrt_profile(None, 0)
        if rc != 0:
            raise RuntimeError(f"axon_start_nrt_profile rc={rc}")
        try:
            yield
        finally:
            n = lib.axon_stop_nrt_profile(str(output_dir).encode())
            print(f"ntff profile: {n} file(s) -> {output_dir}")

    mod = types.ModuleType("antenv.axon_hooks")
    state = {"hook": _hook}
    mod.get_axon_ntff_profile_hook = lambda: state["hook"]
    mod.set_axon_ntff_profile_hook = lambda h: state.update(hook=h)
    sys.modules["antenv.axon_hooks"] = mod
    import antenv

    antenv.axon_hooks = mod


def kernel(x, W_real, W_imag, block_size, out_features):
    global LAST_RESULTS
    x = np.ascontiguousarray(np.asarray(x, dtype=np.float32))
    Wr = np.asarray(W_real, dtype=np.float32)
    Wi = np.asarray(W_imag, dtype=np.float32)
    assert int(block_size) == BS and int(out_features) == D_OUT
    assert x.shape == (B_FULL, D_IN) and Wr.shape == (KOUT, KIN, 129)

    nc = _get_program()
    consts = _build_consts(Wr, Wi)
    core_ids = list(range(NCORES))
    in_maps = [
        {"x": np.ascontiguousarray(x[c * BC:(c + 1) * BC]), **consts}
        for c in core_ids
    ]
    trace = bool(int(os.environ.get("KERNEL_TRACE", "0")))
    if trace:
        _install_ntff_hook()
    res = run_bass_kernel_spmd(nc, in_maps, core_ids, trace=trace)
    LAST_RESULTS = res
    out = np.concatenate([res.results[c]["out"] for c in core_ids], axis=0)
    return np.ascontiguousarray(out.astype(np.float32))


# revision 17
# speedup vs baseline: 1.0426x; 1.0426x over previous
"""Trainium2 Bass kernel for BlockFFTDirectPrior.

Computes out = irfft(einsum('bjn,ijn->bin', rfft(x_blocks), conj(W)))
reshaped to [B, 4096], for x [4096, 4096] f32, W [16, 16, 129] complex
(block size 256).

Strategy: data-parallel over the batch axis across 8 NeuronCores (512 rows
each); W-derived constants replicated. Per core, four PE stages:

  T: transpose x tiles (PE transpose vs identity)      -> xt [t, b] per block
  F: real DFT as matmul (contract t, K=2x128 chunks)   -> X  [n, b] per block
       R0 rows n=0..127 hold Xr[n]; R1 row 0 holds Xr[128] (Nyquist),
       rows p=1..127 hold Xi[p].
  E: per-frequency 16x16 complex mixing as 8-frequency block-diagonal
     matmuls (K = (j,f) = 128)                         -> Y [(i,f), b] per group
  I: real inverse DFT with the data as the stationary operand, which
     restores the [b, m] orientation for free            -> out [b, i*256+m]

Between F/E and E/I, single big SBUF->SBUF DMAs perform the partition
regroupings ((n per j) -> ((j,f) per g), and ((i,f) per g) -> (n per i)).
"""

import os
import numpy as np
from contextlib import ExitStack

import concourse.bass as bass
import concourse.tile as tile
from concourse import bacc, mybir
from concourse.bass_utils import run_bass_kernel_spmd

NCORES = 8
B_FULL, D_IN, D_OUT, BS = 4096, 4096, 4096, 256
BC = B_FULL // NCORES          # 512 batch rows per core
KIN = KOUT = 16
NG = 16                        # groups of 8 frequencies covering n=0..127
F32 = mybir.dt.float32
F32R = mybir.dt.float32r       # single-pass PE fp32 mode (4x faster matmul)

_CACHE = {}
LAST_RESULTS = None            # BassKernelResults of the most recent run


# DFT/IDFT row swizzle: row r = f*16+g holds frequency n = 8g+f. This makes
# both partition regroups plain affine DMAs (partition dim outermost, step 1).
PERM = np.array([8 * (r % 16) + r // 16 for r in range(128)])


def _build_consts(W_real, W_imag):
    """Constant matrices in the exact SBUF layouts the kernel reads."""
    f32 = np.float32
    t = np.arange(BS)
    n0 = np.arange(128)
    ang = 2.0 * np.pi / BS

    CF0 = np.cos(ang * np.outer(t, n0))
    CF1 = np.empty((BS, 128))
    CF1[:, 0] = np.cos(np.pi * t)
    p = np.arange(1, 128)
    CF1[:, 1:] = -np.sin(ang * np.outer(t, p))
    CF0 = CF0[:, PERM]
    CF1 = CF1[:, PERM]
    cfs = np.stack([
        np.concatenate([CF0[:128], CF0[128:]], axis=1),
        np.concatenate([CF1[:128], CF1[128:]], axis=1),
    ], axis=1).astype(f32)                                  # [128, 2, 256]

    # wpk[(j*8+f), g, c, (i*8+f)] = M_c[i, j, 8g+f];  M = (Wr, Wi, -Wi)
    wpk = np.zeros((128, NG, 3, 128), dtype=f32)
    jj = np.arange(KIN)[:, None, None]
    ii = np.arange(KOUT)[None, :, None]
    ff = np.arange(8)[None, None, :]
    for g in range(NG):
        for c, M in enumerate((W_real, W_imag, -W_imag)):
            wpk[jj * 8 + ff, g, c, ii * 8 + ff] = M[ii, jj, 8 * g + ff]
    wnyq = np.ascontiguousarray(W_real[:, :, 128].T).astype(f32)  # [j, i]

    m = np.arange(BS)
    D0 = np.empty((128, BS))
    D0[0] = 1.0 / BS
    nn = np.arange(1, 128)
    D0[1:] = (2.0 / BS) * np.cos(ang * np.outer(nn, m))
    D1 = np.empty((128, BS))
    D1[0] = ((-1.0) ** m) / BS
    D1[1:] = -(2.0 / BS) * np.sin(ang * np.outer(nn, m))
    dmat = np.stack([D0[PERM], D1[PERM]], axis=1).astype(f32)  # [128, 2, 256]

    ident = np.eye(128, dtype=f32)
    return {"cfs": cfs, "wpk": wpk, "wnyq": wnyq, "dmat": dmat, "ident": ident}


def _build_program():
    nc = bacc.Bacc(
        "TRN2", target_bir_lowering=False, debug=False, num_devices=NCORES
    )
    x_d = nc.dram_tensor("x", [BC, D_IN], F32, kind="ExternalInput").ap()
    cfs_d = nc.dram_tensor("cfs", [128, 2, 256], F32R, kind="ExternalInput").ap()
    wpk_d = nc.dram_tensor("wpk", [128, NG, 3, 128], F32R, kind="ExternalInput").ap()
    wnyq_d = nc.dram_tensor("wnyq", [KIN, KOUT], F32R, kind="ExternalInput").ap()
    dmat_d = nc.dram_tensor("dmat", [128, 2, 256], F32R, kind="ExternalInput").ap()
    ident_d = nc.dram_tensor("ident", [128, 128], F32, kind="ExternalInput").ap()
    out_d = nc.dram_tensor("out", [BC, D_OUT], F32, kind="ExternalOutput").ap()

    cp_state = [0]

    with tile.TileContext(nc) as tc, ExitStack() as ctx:
        def copy(dst, src):
            # alternate PSUM->SBUF copies between DVE and ACT
            if cp_state[0] % 2 == 0:
                nc.vector.tensor_copy(dst, src)
            else:
                nc.scalar.copy(dst, src)
            cp_state[0] += 1

        consts = ctx.enter_context(tc.tile_pool(name="consts", bufs=1))
        stg = ctx.enter_context(tc.tile_pool(name="stg", bufs=4))
        gg = ctx.enter_context(tc.tile_pool(name="gg", bufs=2))
        wstr = ctx.enter_context(tc.tile_pool(name="wstr", bufs=4))
        ps = ctx.enter_context(tc.tile_pool(name="ps", bufs=6, space="PSUM"))

        cfs = consts.tile([128, 2, 256], F32R)
        wnyq = consts.tile([KIN, KOUT], F32R)
        dmat = consts.tile([128, 2, 256], F32R)
        ident = consts.tile([128, 128], F32)
        gnyq = consts.tile([KIN, BC], F32R)
        ynyq = consts.tile([KOUT, BC], F32R)

        nc.sync.dma_start(cfs[:], cfs_d)
        nc.sync.dma_start(wnyq[:], wnyq_d)
        nc.sync.dma_start(dmat[:], dmat_d)
        nc.sync.dma_start(ident[:], ident_d)

        # ---- load x: [b, d] in 4 chunks of 128 rows
        xs0 = stg.tile([128, 2, D_IN], F32, tag="stg")
        xs1 = stg.tile([128, 2, D_IN], F32, tag="stg")
        xsv = [xs0, xs1]
        for b4 in range(4):
            nc.sync.dma_start(
                xsv[b4 // 2][:, b4 % 2, :], x_d[128 * b4:128 * (b4 + 1), :]
            )

        # ---- stage T: xt[dc][t_lo, b], dc = j*2 + tc
        xt0 = stg.tile([128, 16, BC], F32R, tag="stg")
        xt1 = stg.tile([128, 16, BC], F32R, tag="stg")
        xtv = [xt0, xt1]
        for bc in range(4):
            for dcg in range(8):
                pt = ps.tile([128, 4, 128], F32, tag="ps")
                for q in range(4):
                    dc = dcg * 4 + q
                    nc.tensor.transpose(
                        pt[:, q, :],
                        xsv[bc // 2][:, bc % 2, 128 * dc:128 * (dc + 1)],
                        ident[:],
                    )
                dst = xtv[dcg // 4][
                    :, 4 * (dcg % 4):4 * (dcg % 4) + 4, 128 * bc:128 * (bc + 1)
                ]
                copy(dst, pt[:])

        # ---- stage F: real DFT (fp32r matmuls); regroup1 DMAs trail per j,
        # alternating issue rings (sync HWDGE / gpsimd SWDGE) for overlap.
        xfr = stg.tile([128, KIN, BC], F32R, tag="stg")
        xfi = stg.tile([128, KIN, BC], F32R, tag="stg")
        ggr = gg.tile([128, NG, BC], F32R, tag="gg")
        ggi = gg.tile([128, NG, BC], F32R, tag="gg")
        for j in range(KIN):
            for which, dstT in ((0, xfr), (1, xfi)):
                pf = ps.tile([128, BC], F32, tag="ps")
                for tc_ in range(2):
                    nc.tensor.matmul(
                        pf[:],
                        cfs[:, which, 128 * tc_:128 * (tc_ + 1)],
                        xtv[j // 8][:, 2 * (j % 8) + tc_, :],
                        start=(tc_ == 0),
                        stop=(tc_ == 1),
                    )
                copy(dstT[:, j, :], pf[:])
            # regroup1: ggr[8j+f, g, b] = xfr[f*16+g, j, b]
            eng = (nc.sync, nc.scalar, nc.gpsimd)[j % 3]
            eng.dma_start(out=ggr[8 * j:8 * (j + 1), :, :], in_=xfr[:, j, :])
            eng.dma_start(out=ggi[8 * j:8 * (j + 1), :, :], in_=xfi[:, j, :])
        nc.sync.dma_start(out=gnyq[:], in_=xfi[0:1, :, :])

        # ---- stage E: blockdiag einsum (fp32r)
        yyr = stg.tile([128, NG, BC], F32R, tag="stg")
        yyi = stg.tile([128, NG, BC], F32R, tag="stg")
        yh0 = gg.tile([128, KOUT, BC], F32R, tag="gg")
        yh1 = gg.tile([128, KOUT, BC], F32R, tag="gg")
        wtiles = []
        for g in range(NG):
            wt = wstr.tile([128, 3, 128], F32R, tag="wstr")
            nc.scalar.dma_start(wt[:], wpk_d[:, g, :, :])
            wtiles.append(wt)
            wt = wtiles[g]
            pyr = ps.tile([128, BC], F32, tag="ps")
            nc.tensor.matmul(pyr[:], wt[:, 0, :],
                             ggr[:, g, :], start=True, stop=False)
            nc.tensor.matmul(pyr[:], wt[:, 1, :],
                             ggi[:, g, :], start=False, stop=True)
            copy(yyr[:, g, :], pyr[:])
            pyi = ps.tile([128, BC], F32, tag="ps")
            nc.tensor.matmul(pyi[:], wt[:, 0, :],
                             ggi[:, g, :], start=True, stop=False)
            nc.tensor.matmul(pyi[:], wt[:, 2, :],
                             ggr[:, g, :], start=False, stop=True)
            copy(yyi[:, g, :], pyi[:])
        pyn = ps.tile([KIN, BC], F32, tag="ps")
        nc.tensor.matmul(pyn[:], wnyq[:],
                         gnyq[:], start=True, stop=True)
        copy(ynyq[:], pyn[:])
        # regroup2: yh0[f*16+g, i, b] = yyr[8i+f, g, b]; same for yh1/yyi
        for i in range(KOUT):
            eng = (nc.sync, nc.scalar, nc.gpsimd)[i % 3]
            eng.dma_start(out=yh0[:, i, :], in_=yyr[8 * i:8 * (i + 1), :, :])
            eng.dma_start(out=yh1[:, i, :], in_=yyi[8 * i:8 * (i + 1), :, :])
        # Nyquist goes to row 0 of yh1 (overwrites the meaningless Zi[0] row)
        nc.sync.dma_start(out=yh1[0:1, :, :], in_=ynyq[:])

        # ---- stage I: inverse DFT, data as stationary operand -> [b, m]
        os0 = stg.tile([128, 2, D_OUT], F32, tag="stg")
        os1 = stg.tile([128, 2, D_OUT], F32, tag="stg")
        osv = [os0, os1]
        for bs in range(4):
            for i in range(KOUT):
                po = ps.tile([128, BS], F32, tag="ps")
                nc.tensor.matmul(
                    po[:], yh0[:, i, 128 * bs:128 * (bs + 1)],
                    dmat[:, 0, :], start=True, stop=False)
                nc.tensor.matmul(
                    po[:], yh1[:, i, 128 * bs:128 * (bs + 1)],
                    dmat[:, 1, :], start=False, stop=True)
                copy(osv[bs // 2][:, bs % 2, BS * i:BS * (i + 1)], po[:])
            nc.sync.dma_start(
                out_d[128 * bs:128 * (bs + 1), :], osv[bs // 2][:, bs % 2, :]
            )

    nc.compile()
    return nc


def _get_program():
    if "nc" not in _CACHE:
        _CACHE["nc"] = _build_program()
    return _CACHE["nc"]


def _install_ntff_hook():
    """Provide antenv.axon_hooks (absent in this image) so that
    run_bass_kernel_spmd(trace=True) can capture NTFF profiles through the
    axon client library."""
    import sys
    import types
    import ctypes
    import contextlib

    if "antenv.axon_hooks" in sys.modules:
        return
    try:
        lib = ctypes.CDLL("/opt/axon/libaxon_pjrt.so")
    except OSError:
        return
    if not hasattr(lib, "axon_start_nrt_profile"):
        return
    lib.axon_start_nrt_profile.argtypes = [
        ctypes.POINTER(ctypes.c_int64),
        ctypes.c_size_t,
    ]
    lib.axon_start_nrt_profile.restype = ctypes.c_int64
    lib.axon_stop_nrt_profile.argtypes = [ctypes.c_char_p]
    lib.axon_stop_nrt_profile.restype = ctypes.c_int64

    @contextlib.contextmanager
    def _hook(output_dir, device_ids):
        import jax

        jax.devices()
        if device_ids:
            ids = (ctypes.c_int64 * len(device_ids))(*device_ids)
            rc = lib.axon_start_nrt_profile(ids, len(device_ids))
        else:
            rc = lib.axon_start_nrt_profile(None, 0)
        if rc != 0:
            raise RuntimeError(f"axon_start_nrt_profile rc={rc}")
        try:
            yield
        finally:
            n = lib.axon_stop_nrt_profile(str(output_dir).encode())
            print(f"ntff profile: {n} file(s) -> {output_dir}")

    mod = types.ModuleType("antenv.axon_hooks")
    state = {"hook": _hook}
    mod.get_axon_ntff_profile_hook = lambda: state["hook"]
    mod.set_axon_ntff_profile_hook = lambda h: state.update(hook=h)
    sys.modules["antenv.axon_hooks"] = mod
    import antenv

    antenv.axon_hooks = mod


def kernel(x, W_real, W_imag, block_size, out_features):
    global LAST_RESULTS
    x = np.ascontiguousarray(np.asarray(x, dtype=np.float32))
    Wr = np.asarray(W_real, dtype=np.float32)
    Wi = np.asarray(W_imag, dtype=np.float32)
    assert int(block_size) == BS and int(out_features) == D_OUT
    assert x.shape == (B_FULL, D_IN) and Wr.shape == (KOUT, KIN, 129)

    nc = _get_program()
    consts = _build_consts(Wr, Wi)
    core_ids = list(range(NCORES))
    in_maps = [
        {"x": np.ascontiguousarray(x[c * BC:(c + 1) * BC]), **consts}
        for c in core_ids
    ]
    trace = bool(int(os.environ.get("KERNEL_TRACE", "0")))
    if trace:
        _install_ntff_hook()
    res = run_bass_kernel_spmd(nc, in_maps, core_ids, trace=trace)
    LAST_RESULTS = res
    out = np.concatenate([res.results[c]["out"] for c in core_ids], axis=0)
    return np.ascontiguousarray(out.astype(np.float32))


# revision 19
# speedup vs baseline: 1.0890x; 1.0445x over previous
"""Trainium2 Bass kernel for BlockFFTDirectPrior.

Computes out = irfft(einsum('bjn,ijn->bin', rfft(x_blocks), conj(W)))
reshaped to [B, 4096], for x [4096, 4096] f32, W [16, 16, 129] complex
(block size 256).

Strategy: data-parallel over the batch axis across 8 NeuronCores (512 rows
each); W-derived constants replicated. Per core, four PE stages:

  T: transpose x tiles (PE transpose vs identity)      -> xt [t, b] per block
  F: real DFT as matmul (contract t, K=2x128 chunks)   -> X  [n, b] per block
       R0 rows n=0..127 hold Xr[n]; R1 row 0 holds Xr[128] (Nyquist),
       rows p=1..127 hold Xi[p].
  E: per-frequency 16x16 complex mixing as 8-frequency block-diagonal
     matmuls (K = (j,f) = 128)                         -> Y [(i,f), b] per group
  I: real inverse DFT with the data as the stationary operand, which
     restores the [b, m] orientation for free            -> out [b, i*256+m]

Between F/E and E/I, single big SBUF->SBUF DMAs perform the partition
regroupings ((n per j) -> ((j,f) per g), and ((i,f) per g) -> (n per i)).
"""

import os
import numpy as np
from contextlib import ExitStack

import concourse.bass as bass
import concourse.tile as tile
from concourse import bacc, mybir
from concourse.bass_utils import run_bass_kernel_spmd

NCORES = 8
B_FULL, D_IN, D_OUT, BS = 4096, 4096, 4096, 256
BC = B_FULL // NCORES          # 512 batch rows per core
KIN = KOUT = 16
NG = 16                        # groups of 8 frequencies covering n=0..127
F32 = mybir.dt.float32
F32R = mybir.dt.float32r       # single-pass PE fp32 mode (4x faster matmul)

_CACHE = {}
LAST_RESULTS = None            # BassKernelResults of the most recent run


# DFT/IDFT row swizzle: row r = f*16+g holds frequency n = 8g+f. This makes
# both partition regroups plain affine DMAs (partition dim outermost, step 1).
PERM = np.array([8 * (r % 16) + r // 16 for r in range(128)])


def _build_consts(W_real, W_imag):
    """Constant matrices in the exact SBUF layouts the kernel reads."""
    f32 = np.float32
    t = np.arange(BS)
    n0 = np.arange(128)
    ang = 2.0 * np.pi / BS

    CF0 = np.cos(ang * np.outer(t, n0))
    CF1 = np.empty((BS, 128))
    CF1[:, 0] = np.cos(np.pi * t)
    p = np.arange(1, 128)
    CF1[:, 1:] = -np.sin(ang * np.outer(t, p))
    CF0 = CF0[:, PERM]
    CF1 = CF1[:, PERM]
    cfs = np.stack([
        np.concatenate([CF0[:128], CF0[128:]], axis=1),
        np.concatenate([CF1[:128], CF1[128:]], axis=1),
    ], axis=1).astype(f32)                                  # [128, 2, 256]

    # wpk[(f*16+j), g, c, (f*16+i)] = M_c[i, j, 8g+f];  M = (Wr, Wi, -Wi)
    wpk = np.zeros((128, NG, 3, 128), dtype=f32)
    jj = np.arange(KIN)[:, None, None]
    ii = np.arange(KOUT)[None, :, None]
    ff = np.arange(8)[None, None, :]
    for g in range(NG):
        for c, M in enumerate((W_real, W_imag, -W_imag)):
            wpk[ff * 16 + jj, g, c, ff * 16 + ii] = M[ii, jj, 8 * g + ff]
    wnyq = np.ascontiguousarray(W_real[:, :, 128].T).astype(f32)  # [j, i]

    m = np.arange(BS)
    D0 = np.empty((128, BS))
    D0[0] = 1.0 / BS
    nn = np.arange(1, 128)
    D0[1:] = (2.0 / BS) * np.cos(ang * np.outer(nn, m))
    D1 = np.empty((128, BS))
    D1[0] = ((-1.0) ** m) / BS
    D1[1:] = -(2.0 / BS) * np.sin(ang * np.outer(nn, m))
    dmat = np.stack([D0[PERM], D1[PERM]], axis=1).astype(f32)  # [128, 2, 256]

    ident = np.eye(128, dtype=f32)
    return {"cfs": cfs, "wpk": wpk, "wnyq": wnyq, "dmat": dmat, "ident": ident}


def _build_program():
    nc = bacc.Bacc(
        "TRN2", target_bir_lowering=False, debug=False, num_devices=NCORES
    )
    x_d = nc.dram_tensor("x", [BC, D_IN], F32, kind="ExternalInput").ap()
    cfs_d = nc.dram_tensor("cfs", [128, 2, 256], F32R, kind="ExternalInput").ap()
    wpk_d = nc.dram_tensor("wpk", [128, NG, 3, 128], F32R, kind="ExternalInput").ap()
    wnyq_d = nc.dram_tensor("wnyq", [KIN, KOUT], F32R, kind="ExternalInput").ap()
    dmat_d = nc.dram_tensor("dmat", [128, 2, 256], F32R, kind="ExternalInput").ap()
    ident_d = nc.dram_tensor("ident", [128, 128], F32, kind="ExternalInput").ap()
    out_d = nc.dram_tensor("out", [BC, D_OUT], F32, kind="ExternalOutput").ap()

    cp_state = [0]

    with tile.TileContext(nc) as tc, ExitStack() as ctx:
        def copy(dst, src):
            # alternate PSUM->SBUF copies between DVE and ACT
            if cp_state[0] % 2 == 0:
                nc.vector.tensor_copy(dst, src)
            else:
                nc.scalar.copy(dst, src)
            cp_state[0] += 1

        consts = ctx.enter_context(tc.tile_pool(name="consts", bufs=1))
        stg = ctx.enter_context(tc.tile_pool(name="stg", bufs=5))
        ps = ctx.enter_context(tc.tile_pool(name="ps", bufs=6, space="PSUM"))

        cfs = consts.tile([128, 2, 256], F32R)
        wpk = consts.tile([128, NG, 3, 128], F32R)
        wnyq = consts.tile([KIN, KOUT], F32R)
        dmat = consts.tile([128, 2, 256], F32R)
        ident = consts.tile([128, 128], F32)
        gnyq = consts.tile([KIN, BC], F32R)
        ynyq = consts.tile([KOUT, BC], F32R)

        nc.sync.dma_start(cfs[:], cfs_d)
        nc.sync.dma_start(wpk[:], wpk_d)
        nc.sync.dma_start(wnyq[:], wnyq_d)
        nc.sync.dma_start(dmat[:], dmat_d)
        nc.sync.dma_start(ident[:], ident_d)

        # ---- load x: [b, d] in 4 chunks of 128 rows
        xs0 = stg.tile([128, 2, D_IN], F32, tag="stg")
        xs1 = stg.tile([128, 2, D_IN], F32, tag="stg")
        xsv = [xs0, xs1]
        for b4 in range(4):
            nc.sync.dma_start(
                xsv[b4 // 2][:, b4 % 2, :], x_d[128 * b4:128 * (b4 + 1), :]
            )

        # ---- stage T: xt[dc][t_lo, b], dc = j*2 + tc
        xt0 = stg.tile([128, 16, BC], F32R, tag="stg")
        xt1 = stg.tile([128, 16, BC], F32R, tag="stg")
        xtv = [xt0, xt1]
        for bc in range(4):
            for dcg in range(8):
                pt = ps.tile([128, 4, 128], F32, tag="ps")
                for q in range(4):
                    dc = dcg * 4 + q
                    nc.tensor.transpose(
                        pt[:, q, :],
                        xsv[bc // 2][:, bc % 2, 128 * dc:128 * (dc + 1)],
                        ident[:],
                    )
                dst = xtv[dcg // 4][
                    :, 4 * (dcg % 4):4 * (dcg % 4) + 4, 128 * bc:128 * (bc + 1)
                ]
                copy(dst, pt[:])

        # ---- stage F: real DFT (fp32r matmuls); regroup1 DMAs trail per j,
        # alternating issue rings (sync HWDGE / gpsimd SWDGE) for overlap.
        xfr = stg.tile([128, KIN, BC], F32R, tag="stg")
        xfi = stg.tile([128, KIN, BC], F32R, tag="stg")
        ggr = stg.tile([128, NG, BC], F32R, tag="stg")
        ggi = stg.tile([128, NG, BC], F32R, tag="stg")
        for j in range(KIN):
            for which, dstT in ((0, xfr), (1, xfi)):
                pf = ps.tile([128, BC], F32, tag="ps")
                for tc_ in range(2):
                    nc.tensor.matmul(
                        pf[:],
                        cfs[:, which, 128 * tc_:128 * (tc_ + 1)],
                        xtv[j // 8][:, 2 * (j % 8) + tc_, :],
                        start=(tc_ == 0),
                        stop=(tc_ == 1),
                    )
                copy(dstT[:, j, :], pf[:])
        # regroup1 (per g, so stage E can start per group):
        # ggr[(f*16+j), g, b] = xfr[f*16+g, j, b]
        for g in range(NG):
            eng = nc.sync if g % 2 == 0 else nc.gpsimd
            eng.dma_start(out=ggr[:, g, :], in_=xfr[g::16, :, :])
            eng.dma_start(out=ggi[:, g, :], in_=xfi[g::16, :, :])
        nc.scalar.dma_start(out=gnyq[:], in_=xfi[0:1, :, :])

        # ---- stage E: blockdiag einsum (fp32r)
        yyr = stg.tile([128, NG, BC], F32R, tag="stg")
        yyi = stg.tile([128, NG, BC], F32R, tag="stg")
        yh0 = stg.tile([128, KOUT, BC], F32R, tag="stg")
        yh1 = stg.tile([128, KOUT, BC], F32R, tag="stg")
        for g in range(NG):
            pyr = ps.tile([128, BC], F32, tag="ps")
            nc.tensor.matmul(pyr[:], wpk[:, g, 0, :],
                             ggr[:, g, :], start=True, stop=False)
            nc.tensor.matmul(pyr[:], wpk[:, g, 1, :],
                             ggi[:, g, :], start=False, stop=True)
            copy(yyr[:, g, :], pyr[:])
            pyi = ps.tile([128, BC], F32, tag="ps")
            nc.tensor.matmul(pyi[:], wpk[:, g, 0, :],
                             ggi[:, g, :], start=True, stop=False)
            nc.tensor.matmul(pyi[:], wpk[:, g, 2, :],
                             ggr[:, g, :], start=False, stop=True)
            copy(yyi[:, g, :], pyi[:])
        pyn = ps.tile([KIN, BC], F32, tag="ps")
        nc.tensor.matmul(pyn[:], wnyq[:],
                         gnyq[:], start=True, stop=True)
        copy(ynyq[:], pyn[:])
        # regroup2: yh0[f*16+g, i, b] = yyr[8i+f, g, b]; same for yh1/yyi
        for i in range(KOUT):
            eng = nc.sync if i % 2 == 0 else nc.gpsimd
            eng.dma_start(out=yh0[:, i, :], in_=yyr[i::16, :, :])
            eng.dma_start(out=yh1[:, i, :], in_=yyi[i::16, :, :])
        # Nyquist goes to row 0 of yh1 (overwrites the meaningless Zi[0] row)
        nc.sync.dma_start(out=yh1[0:1, :, :], in_=ynyq[:])

        # ---- stage I: inverse DFT, data as stationary operand -> [b, m]
        os0 = stg.tile([128, 2, D_OUT], F32, tag="stg")
        os1 = stg.tile([128, 2, D_OUT], F32, tag="stg")
        osv = [os0, os1]
        for bs in range(4):
            for i in range(KOUT):
                po = ps.tile([128, BS], F32, tag="ps")
                nc.tensor.matmul(
                    po[:], yh0[:, i, 128 * bs:128 * (bs + 1)],
                    dmat[:, 0, :], start=True, stop=False)
                nc.tensor.matmul(
                    po[:], yh1[:, i, 128 * bs:128 * (bs + 1)],
                    dmat[:, 1, :], start=False, stop=True)
                copy(osv[bs // 2][:, bs % 2, BS * i:BS * (i + 1)], po[:])
            nc.sync.dma_start(
                out_d[128 * bs:128 * (bs + 1), :], osv[bs // 2][:, bs % 2, :]
            )

    nc.compile()
    return nc


def _get_program():
    if "nc" not in _CACHE:
        _CACHE["nc"] = _build_program()
    return _CACHE["nc"]


def _install_ntff_hook():
    """Provide antenv.axon_hooks (absent in this image) so that
    run_bass_kernel_spmd(trace=True) can capture NTFF profiles through the
    axon client library."""
    import sys
    import types
    import ctypes
    import contextlib

    if "antenv.axon_hooks" in sys.modules:
        return
    try:
        lib = ctypes.CDLL("/opt/axon/libaxon_pjrt.so")
    except OSError:
        return
    if not hasattr(lib, "axon_start_nrt_profile"):
        return
    lib.axon_start_nrt_profile.argtypes = [
        ctypes.POINTER(ctypes.c_int64),
        ctypes.c_size_t,
    ]
    lib.axon_start_nrt_profile.restype = ctypes.c_int64
    lib.axon_stop_nrt_profile.argtypes = [ctypes.c_char_p]
    lib.axon_stop_nrt_profile.restype = ctypes.c_int64

    @contextlib.contextmanager
    def _hook(output_dir, device_ids):
        import jax

        jax.devices()
        if device_ids:
            ids = (ctypes.c_int64 * len(device_ids))(*device_ids)
            rc = lib.axon_start_nrt_profile(ids, len(device_ids))
        else:
            rc = lib.axon_start_nrt_profile(None, 0)
        if rc != 0:
            raise RuntimeError(f"axon_start_nrt_profile rc={rc}")
        try:
            yield
        finally:
            n = lib.axon_stop_nrt_profile(str(output_dir).encode())
            print(f"ntff profile: {n} file(s) -> {output_dir}")

    mod = types.ModuleType("antenv.axon_hooks")
    state = {"hook": _hook}
    mod.get_axon_ntff_profile_hook = lambda: state["hook"]
    mod.set_axon_ntff_profile_hook = lambda h: state.update(hook=h)
    sys.modules["antenv.axon_hooks"] = mod
    import antenv

    antenv.axon_hooks = mod


def kernel(x, W_real, W_imag, block_size, out_features):
    global LAST_RESULTS
    x = np.ascontiguousarray(np.asarray(x, dtype=np.float32))
    Wr = np.asarray(W_real, dtype=np.float32)
    Wi = np.asarray(W_imag, dtype=np.float32)
    assert int(block_size) == BS and int(out_features) == D_OUT
    assert x.shape == (B_FULL, D_IN) and Wr.shape == (KOUT, KIN, 129)

    nc = _get_program()
    consts = _build_consts(Wr, Wi)
    core_ids = list(range(NCORES))
    in_maps = [
        {"x": np.ascontiguousarray(x[c * BC:(c + 1) * BC]), **consts}
        for c in core_ids
    ]
    trace = bool(int(os.environ.get("KERNEL_TRACE", "0")))
    if trace:
        _install_ntff_hook()
    res = run_bass_kernel_spmd(nc, in_maps, core_ids, trace=trace)
    LAST_RESULTS = res
    out = np.concatenate([res.results[c]["out"] for c in core_ids], axis=0)
    return np.ascontiguousarray(out.astype(np.float32))
